# revision 14
# baseline (speedup 1.0000x reference)
"""CombinedCSA (channel+spatial attention) Trainium2 Bass kernel.

Sharding: data-parallel over batch. 16 images / 8 cores = 2 images per core.
Weights (fc1/fc2/conv) replicated, pre-transposed host-side.

v4 dataflow (f32-resident chpool, bf16 downstream, software-pipelined):
  per chunk (phase A): load f32 (sync HWDGE) into resident ring;
    DVE channel-max reduce (independent per chunk); ACT in-place copy
    whose accum_out yields the channel sum
  MLP (PE matmuls, ACT relu/sigmoid) -> per-channel scale
  per chunk (phase C):
    xs[h] = ACT scale-convert (f32 -> bf16, scale=sigmoid scale_c) --
      one ACT op per half doing scale AND bf16 conversion
    spatial max: DVE bf16 max(xs0,xs1) -> PE bf16 transposes -> DVE
      segmented max-reduce into smaxT
    spatial sum: PE [128x128]@[128x1] ones-matmuls into savgT psum
    7x7 conv: 14 banded matmuls on PE (bands built host-side)
    sigmoid -> PE transpose -> row-collapse DMA (scalar ring) -> Pool bcast
    final: DVE bf16 tensor_mul IN PLACE on xs -> SWDGE cast-store bf16->f32
  Emission interleaves image b+1's phase A into image b's phase C so each
  in-order engine stream can advance both pipelines.
"""

import os
import numpy as np
from contextlib import ExitStack

import concourse.bass as bass
import concourse.tile as tile
from concourse import bacc, mybir
from concourse._compat import with_exitstack
from concourse.bass_utils import run_bass_kernel_spmd

F32 = mybir.dt.float32
BF16 = mybir.dt.bfloat16
AF = mybir.ActivationFunctionType
AX = mybir.AxisListType
OP = mybir.AluOpType

# tuning knobs (env-overridable for experiments)
XP_BUFS = int(os.environ.get("CSA_XP", "19"))
XS_BUFS = int(os.environ.get("CSA_XS", "3"))
BC_BUFS = int(os.environ.get("CSA_BC", "2"))
TPP_BUFS = int(os.environ.get("CSA_TPP", "2"))
STORE_CAST = os.environ.get("CSA_STORE", "cast") == "cast"
# conv-group schedule: list of (emit_k, k0, k1)
_SCHEDS = {
    "A": [(2, 0, 2), (4, 2, 4), (6, 4, 6), (7, 6, 8)],
    "D": [(2, 0, 2), (4, 2, 4), (6, 4, 6), (7, 6, 7), (7, 7, 8)],
}
CONV_SCHED = _SCHEDS[os.environ.get("CSA_SCHED", "A")]

# Problem constants (hardcoded; see spec)
B, C, H, W = 16, 256, 128, 128
HW = H * W          # 16384
R = 16              # Cr = C // 16
NCORES = 8
BLOC = B // NCORES  # 2 images per core
NH = 2              # channel halves of 128
P = 128
FCH = 2048          # hw elements per chunk (16 h-rows)
NCH = HW // FCH     # 8 chunks per image
HROWS = FCH // W    # 16 h-rows per chunk
CONVG = 2           # chunks per conv group
NBLK = FCH // P     # 16 transpose blocks per chunk


@with_exitstack
def csa_kernel(ctx, tc, out_d, x_d, w1t_d, w2t_d, bands_d, ident_d,
               skip=frozenset()):
    nc = tc.nc

    # ---- pools ----
    xp = ctx.enter_context(tc.tile_pool(name="xp", bufs=XP_BUFS))
    xsp = ctx.enter_context(tc.tile_pool(name="xsp", bufs=XS_BUFS))
    xmp = ctx.enter_context(tc.tile_pool(name="xmp", bufs=int(os.environ.get("CSA_XM", "1"))))
    bcp = ctx.enter_context(tc.tile_pool(name="bcp", bufs=BC_BUFS))
    rowp = ctx.enter_context(tc.tile_pool(name="rowp", bufs=1))
    stat = ctx.enter_context(tc.tile_pool(name="stat", bufs=2))
    cons = ctx.enter_context(tc.tile_pool(name="cons", bufs=1))
    tpp = ctx.enter_context(tc.tile_pool(name="tpp", bufs=TPP_BUFS, space="PSUM"))
    savgpp = ctx.enter_context(tc.tile_pool(name="savgpp", bufs=1, space="PSUM"))
    convp = ctx.enter_context(tc.tile_pool(name="convp", bufs=2, space="PSUM"))
    atpp = ctx.enter_context(tc.tile_pool(name="atpp", bufs=1, space="PSUM"))
    mlpp = ctx.enter_context(tc.tile_pool(name="mlpp", bufs=1, space="PSUM"))

    # ---- constants / weights ----
    w1t_sb = cons.tile([P, NH * R], F32)           # [128, 32]: col block h = w_fc1.T half h
    for h in range(NH):
        nc.sync.dma_start(out=w1t_sb[:, h * R:(h + 1) * R],
                          in_=w1t_d[h * P:(h + 1) * P, :])
    w2t_sb = cons.tile([R, C], F32)                # [16, 256] = w_fc2.T
    nc.sync.dma_start(out=w2t_sb[:], in_=w2t_d[:])
    bands_sb = cons.tile([P, 14 * P], F32)         # [128, (ci, w)]
    nc.sync.dma_start(out=bands_sb[:].rearrange("p (c w) -> p c w", c=14),
                      in_=bands_d.transpose([1, 0, 2]))
    ident_f = cons.tile([P, P], F32)
    nc.sync.dma_start(out=ident_f[:], in_=ident_d[:])
    ident_bf = cons.tile([P, P], BF16)
    nc.scalar.activation(out=ident_bf[:], in_=ident_f[:], func=AF.Copy)
    ones_bf = cons.tile([P, 1], BF16)
    nc.vector.memset(ones_bf[:], 1.0)

    # ---- per-image state ----
    xt = [[[None] * NCH for _ in range(NH)] for _ in range(BLOC)]
    xs_t = [[[None] * NCH for _ in range(NH)] for _ in range(BLOC)]
    chmax = [[None] * NH for _ in range(BLOC)]
    chsum = [[None] * NH for _ in range(BLOC)]
    scale_sb = [[None] * NH for _ in range(BLOC)]
    smaxT = [None] * BLOC
    savgT = [None] * BLOC
    savg_ps = [None] * BLOC
    conv_ps = [None] * BLOC
    bcs = [{} for _ in range(BLOC)]

    def phase_a_chunk(b, k):
        """Load chunk k of image b; channel max (DVE) + sum (ACT accum)."""
        if k == 0:
            for h in range(NH):
                chmax[b][h] = stat.tile([P, NCH], F32, name=f"chmax{b}{h}",
                                        tag=f"chmax{h}")
                chsum[b][h] = stat.tile([P, NCH], F32, name=f"chsum{b}{h}",
                                        tag=f"chsum{h}")
        for h in range(NH):
            t = xp.tile([P, FCH], F32, name=f"x{b}{h}{k}", tag="x",
                        bufs=XP_BUFS)
            xt[b][h][k] = t
            nc.sync.dma_start(
                out=t[:],
                in_=x_d[b, h * P:(h + 1) * P, k * FCH:(k + 1) * FCH])
            if "chpool" in skip:
                continue
            nc.vector.tensor_reduce(
                out=chmax[b][h][:, k:k + 1], in_=t[:],
                axis=AX.X, op=OP.max)
            # in-place copy whose only purpose is the free-dim sum output
            nc.scalar.activation(out=t[:], in_=t[:], func=AF.Copy,
                                 accum_out=chsum[b][h][:, k:k + 1])

    def mlp(b):
        z_ps = mlpp.tile([R, 1], F32, name=f"zps{b}", tag="mlp")
        hvec = []
        for h in range(NH):
            cmf = stat.tile([P, 1], F32, name=f"chmaxf{b}{h}", tag=f"chmaxf{h}")
            csf = stat.tile([P, 1], F32, name=f"chsumf{b}{h}", tag=f"chsumf{h}")
            if "chpool" in skip:
                nc.vector.memset(cmf[:], 0.5)
                nc.vector.memset(csf[:], 0.5)
            else:
                nc.vector.tensor_reduce(out=cmf[:], in_=chmax[b][h][:],
                                        axis=AX.X, op=OP.max)
                nc.vector.tensor_reduce(out=csf[:], in_=chsum[b][h][:],
                                        axis=AX.X, op=OP.add)
            hv = stat.tile([P, 1], F32, name=f"hvec{b}{h}", tag=f"hvec{h}")
            # hv = chmax + chsum/HW
            nc.scalar.activation(out=hv[:], in_=csf[:], func=AF.Identity,
                                 bias=cmf[:, 0:1], scale=1.0 / HW)
            hvec.append(hv)
        for h in range(NH):
            nc.tensor.matmul(out=z_ps[:], lhsT=w1t_sb[:, h * R:(h + 1) * R],
                             rhs=hvec[h][:], start=(h == 0), stop=(h == NH - 1))
        zr = stat.tile([R, 1], F32, name=f"zrelu{b}", tag="zrelu")
        nc.scalar.activation(out=zr[:], in_=z_ps[:], func=AF.Relu)
        for h in range(NH):
            l_ps = mlpp.tile([P, 1], F32, name=f"lps{b}{h}", tag="mlp")
            nc.tensor.matmul(out=l_ps[:], lhsT=w2t_sb[:, h * P:(h + 1) * P],
                             rhs=zr[:], start=True, stop=True)
            sc = stat.tile([P, 1], F32, name=f"scale{b}{h}", tag=f"scale{h}")
            nc.scalar.activation(out=sc[:], in_=l_ps[:], func=AF.Sigmoid)
            scale_sb[b][h] = sc
        # per-image spatial-stat state
        smaxT[b] = stat.tile([P, H], F32, name=f"smaxT{b}", tag="smaxT")
        savgT[b] = stat.tile([P, H], F32, name=f"savgT{b}", tag="savgT")
        savg_ps[b] = savgpp.tile([P, H], F32, name=f"savgps{b}", tag="savg")
        conv_ps[b] = convp.tile([P, H], F32, name=f"convps{b}", tag="conv")
        if "trans" in skip:
            nc.vector.memset(smaxT[b][:], 0.25)
        if "savg" in skip:
            nc.vector.memset(savgT[b][:], 0.25)

    def conv_pair(b, k0, k1):
        h0c, h1c = k0 * HROWS, k1 * HROWS
        # 7x7 conv as banded matmuls: out[:, h] += bandT_{c,i} @ statT[:, h+i-3]
        mms = []
        for c, st in ((0, smaxT[b]), (1, savgT[b])):
            for i in range(7):
                lo = max(h0c, 3 - i)
                hi = min(h1c, H + 3 - i)
                if lo >= hi:
                    continue
                mms.append((c, i, lo, hi, st))
        # identity-shift tap first so start=True covers the whole column range
        mms.sort(key=lambda m: (m[1] != 3 or m[0] != 0))
        for n, (c, i, lo, hi, st) in enumerate(mms):
            assert not (n == 0 and (lo != h0c or hi != h1c))
            nc.tensor.matmul(
                out=conv_ps[b][:, lo:hi],
                lhsT=bands_sb[:, (c * 7 + i) * P:(c * 7 + i + 1) * P],
                rhs=st[:, lo + i - 3:hi + i - 3],
                start=(n == 0), stop=(n == len(mms) - 1),
                skip_group_check=True)

    def attn_chunk(b, kc):
        h0c, h1c = kc * HROWS, (kc + 1) * HROWS
        attn_wh = stat.tile([P, HROWS], BF16, name=f"attnwh{b}{kc}",
                            tag="attnwh", bufs=3)
        nc.scalar.activation(out=attn_wh[:], in_=conv_ps[b][:, h0c:h1c],
                             func=AF.Sigmoid)
        at_ps = atpp.tile([HROWS, P], BF16, name=f"atps{b}{kc}", tag="atp")
        nc.tensor.transpose(out=at_ps[:], in_=attn_wh[:],
                            identity=ident_bf[:])
        attn_hw = stat.tile([HROWS, P], BF16, name=f"attnhw{b}{kc}",
                            tag="attnhw", bufs=3)
        nc.scalar.activation(out=attn_hw[:], in_=at_ps[:], func=AF.Copy)
        row = rowp.tile([1, FCH], BF16, name=f"row{b}{kc}", tag="row")
        nc.scalar.dma_start(
            out=row[:].rearrange("p (h w) -> p h w", h=HROWS),
            in_=attn_hw[:])
        bc = bcp.tile([P, FCH], BF16, name=f"bc{b}{kc}", tag="bc",
                      bufs=BC_BUFS)
        nc.gpsimd.partition_broadcast(bc[:], row[:], channels=P)
        bcs[b][kc] = bc

    def conv_attn(b, k0, k1):
        if "conv" not in skip:
            conv_pair(b, k0, k1)
            for kc in range(k0, k1):
                attn_chunk(b, kc)

    def finals(b, k0, k1):
        for kc in range(k0, k1):
            for h in range(NH):
                xs = xs_t[b][h][kc]
                if "final" not in skip and "conv" not in skip:
                    nc.vector.tensor_mul(xs[:], xs[:], bcs[b][kc][:])
                dst = out_d[b, h * P:(h + 1) * P, kc * FCH:(kc + 1) * FCH]
                if STORE_CAST:
                    nc.gpsimd.dma_start(out=dst, in_=xs[:])
                else:
                    nc.scalar.dma_start(out=dst, in_=xs[:])

    def phase_c_chunk(b, k):
        # scale-convert both halves: xs = bf16(x * scale_c), one ACT op each
        for h in range(NH):
            xs = xsp.tile([P, FCH], BF16, name=f"xs{b}{h}{k}", tag=f"xs{h}",
                          bufs=XS_BUFS)
            xs_t[b][h][k] = xs
            if "scale" in skip:
                nc.scalar.activation(out=xs[:], in_=xt[b][h][k][:],
                                     func=AF.Copy)
            else:
                nc.scalar.activation(out=xs[:], in_=xt[b][h][k][:],
                                     func=AF.Copy,
                                     scale=scale_sb[b][h][:, 0:1])
        xs0, xs1 = xs_t[b][0][k], xs_t[b][1][k]
        if "trans" not in skip:
            # spatial max over C: combine halves (bf16 2x), PE transpose,
            # segmented max-reduce into smaxT columns
            xm = xmp.tile([P, FCH], BF16, name=f"xm{b}{k}", tag="xm")
            nc.vector.tensor_max(xm[:], xs0[:], xs1[:])
            for j8 in range(NBLK // 8):
                tpt = tpp.tile([P, 8 * P], BF16, name=f"tp{b}{k}{j8}",
                               tag="tp")
                for jj in range(8):
                    j = j8 * 8 + jj
                    nc.tensor.transpose(out=tpt[:, jj * P:(jj + 1) * P],
                                        in_=xm[:, j * P:(j + 1) * P],
                                        identity=ident_bf[:])
                g0 = k * NBLK + j8 * 8
                nc.vector.tensor_reduce(
                    out=smaxT[b][:, g0:g0 + 8],
                    in_=tpt[:].rearrange("p (b f) -> p b f", b=8),
                    axis=AX.X, op=OP.max)
        if "savg" not in skip:
            # spatial sum over C: per 128-px block, ones-matmul accumulating
            # both halves into savg_ps column k*16+j
            for j in range(NBLK):
                col = k * NBLK + j
                nc.tensor.matmul(out=savg_ps[b][:, col:col + 1],
                                 lhsT=xs0[:, j * P:(j + 1) * P],
                                 rhs=ones_bf[:], start=True, stop=False,
                                 skip_group_check=True)
                nc.tensor.matmul(out=savg_ps[b][:, col:col + 1],
                                 lhsT=xs1[:, j * P:(j + 1) * P],
                                 rhs=ones_bf[:], start=False, stop=True,
                                 skip_group_check=True)
            nc.scalar.activation(
                out=savgT[b][:, k * NBLK:(k + 1) * NBLK],
                in_=savg_ps[b][:, k * NBLK:(k + 1) * NBLK], func=AF.Copy)
        for (ek, k0, k1) in CONV_SCHED:
            if ek == k:
                conv_attn(b, k0, k1)
                finals(b, k0, k1)

    # ---- software-pipelined emission ----
    for k in range(NCH):
        phase_a_chunk(0, k)
    mlp(0)
    for k in range(NCH):
        phase_c_chunk(0, k)
        if BLOC > 1:
            phase_a_chunk(1, k)
    if BLOC > 1:
        mlp(1)
        for k in range(NCH):
            phase_c_chunk(1, k)


def _build_nc(reps: int = 1, skip=frozenset()):
    nc = bacc.Bacc("TRN2", target_bir_lowering=False, debug=False,
                   num_devices=NCORES)
    x_d = nc.dram_tensor("x", [BLOC, C, HW], F32, kind="ExternalInput").ap()
    w1t_d = nc.dram_tensor("w1t", [C, R], F32, kind="ExternalInput").ap()
    w2t_d = nc.dram_tensor("w2t", [R, C], F32, kind="ExternalInput").ap()
    bands_d = nc.dram_tensor("bands", [14, W, W], F32, kind="ExternalInput").ap()
    ident_d = nc.dram_tensor("ident", [P, P], F32, kind="ExternalInput").ap()
    out_d = nc.dram_tensor("out", [BLOC, C, HW], F32, kind="ExternalOutput").ap()
    with tile.TileContext(nc) as tc:
        for _ in range(reps):
            csa_kernel(tc, out_d, x_d, w1t_d, w2t_d, bands_d, ident_d,
                       skip=skip)
    nc.compile()
    return nc


_NC_CACHE = None


def _get_nc():
    global _NC_CACHE
    if _NC_CACHE is None:
        _NC_CACHE = _build_nc()
    return _NC_CACHE


def build_bands(w_conv):
    """[14, W, W] transposed band matrices; bands[c*7+i][w', w] =
    w_conv[0, c, i, w'-w+3]; avg channel folded with 1/C."""
    w_conv = np.asarray(w_conv, np.float32)
    bands = np.zeros((2, 7, W, W), np.float32)
    for c in range(2):
        for i in range(7):
            for kj in range(7):
                bands[c, i] += w_conv[0, c, i, kj] * np.eye(W, k=3 - kj,
                                                            dtype=np.float32)
    bands[1] /= C
    return bands.reshape(14, W, W)


def make_in_maps(x, w_fc1, w_fc2, w_conv):
    x = np.ascontiguousarray(np.asarray(x, np.float32))
    w1t = np.ascontiguousarray(np.asarray(w_fc1, np.float32).T)
    w2t = np.ascontiguousarray(np.asarray(w_fc2, np.float32).T)
    bands = build_bands(w_conv)
    ident = np.eye(P, dtype=np.float32)
    xr = x.reshape(NCORES, BLOC, C, HW)
    return [{"x": np.ascontiguousarray(xr[i]), "w1t": w1t, "w2t": w2t,
             "bands": bands, "ident": ident} for i in range(NCORES)]


def kernel(x, w_fc1, w_fc2, w_conv):
    nc = _get_nc()
    in_maps = make_in_maps(x, w_fc1, w_fc2, w_conv)
    res = run_bass_kernel_spmd(nc, in_maps, list(range(NCORES)))
    out = np.stack([res.results[i]["out"] for i in range(NCORES)])
    return out.reshape(B, C, H, W).astype(np.float32)


# revision 15
# speedup vs baseline: 1.2482x; 1.2482x over previous
"""CombinedCSA (channel+spatial attention) Trainium2 Bass kernel.

Sharding: data-parallel over batch. 16 images / 8 cores = 2 images per core.
Weights (fc1/fc2/conv) replicated, pre-transposed host-side.

Per-core dataflow (per image, streamed in HW chunks of 16 rows):
  load chunk -> channel-max (DVE reduce) + channel-sum (ACT accum_out)
  MLP (PE matmuls + ACT relu/sigmoid) -> per-channel scale
  scale chunk in place (ACT, per-partition scale)
  spatial max over C: DVE max(half0,half1) -> PE transpose -> DVE segmented reduce
  spatial sum over C: PE matmul (x block stationary, ones moving)
  7x7 conv: 14 banded matmuls on PE (bands built host-side)
  sigmoid -> transpose -> row-collapse DMA -> gpsimd partition_broadcast
  final multiply in place (DVE / gpsimd split) -> store
"""

import os
import numpy as np
from contextlib import ExitStack

import concourse.bass as bass
import concourse.tile as tile
from concourse import bacc, mybir
from concourse._compat import with_exitstack
from concourse.bass_utils import run_bass_kernel_spmd

F32 = mybir.dt.float32
AF = mybir.ActivationFunctionType

# Problem constants (hardcoded; see spec)
B, C, H, W = 16, 256, 128, 128
HW = H * W          # 16384
R = 16              # Cr = C // 16
NCORES = 8
BLOC = B // NCORES  # 2 images per core
NH = 2              # channel halves of 128
P = 128
FCH = 2048          # hw elements per chunk (16 h-rows)
NCH = HW // FCH     # 8 chunks per image
HROWS = FCH // W    # 16 h-rows per chunk
CONVG = 2           # chunks per conv group
NBLK = FCH // P     # 16 transpose blocks per chunk

# chunk indices whose heavy elementwise ops go to gpsimd instead of DVE
# (walrus rejects TensorTensor on the Pool engine on this toolchain, so empty)
GPS_FINAL = frozenset()
GPS_COMBINE = frozenset()


@with_exitstack
def csa_kernel(ctx, tc, out_d, x_d, w1t_d, w2t_d, bands_d, ident_d,
               skip=frozenset()):
    nc = tc.nc

    # ---- pools ----
    xp = ctx.enter_context(tc.tile_pool(name="xp", bufs=19))
    xmaxp = ctx.enter_context(tc.tile_pool(name="xmaxp", bufs=2))
    bcp = ctx.enter_context(tc.tile_pool(name="bcp", bufs=1))
    rowp = ctx.enter_context(tc.tile_pool(name="rowp", bufs=1))
    stat = ctx.enter_context(tc.tile_pool(name="stat", bufs=2))
    cons = ctx.enter_context(tc.tile_pool(name="cons", bufs=1))
    tp = ctx.enter_context(tc.tile_pool(name="tp", bufs=2, space="PSUM"))
    tsp = ctx.enter_context(tc.tile_pool(name="tsp", bufs=2, space="PSUM"))
    convp = ctx.enter_context(tc.tile_pool(name="convp", bufs=2, space="PSUM"))
    atpp = ctx.enter_context(tc.tile_pool(name="atpp", bufs=1, space="PSUM"))
    mlpp = ctx.enter_context(tc.tile_pool(name="mlpp", bufs=1, space="PSUM"))

    # ---- constants / weights ----
    w1t_sb = cons.tile([P, NH * R], F32)           # [128, 32]: col block h = w_fc1.T half h
    for h in range(NH):
        nc.sync.dma_start(out=w1t_sb[:, h * R:(h + 1) * R],
                          in_=w1t_d[h * P:(h + 1) * P, :])
    w2t_sb = cons.tile([R, C], F32)                # [16, 256] = w_fc2.T
    nc.sync.dma_start(out=w2t_sb[:], in_=w2t_d[:])
    bands_sb = cons.tile([P, 14 * P], F32)         # [128, (ci, w)]
    nc.sync.dma_start(out=bands_sb[:].rearrange("p (c w) -> p c w", c=14),
                      in_=bands_d.transpose([1, 0, 2]))
    ident_sb = cons.tile([P, P], F32)
    nc.sync.dma_start(out=ident_sb[:], in_=ident_d[:])
    ones_sb = cons.tile([P, 1], F32)
    nc.vector.memset(ones_sb[:], 1.0)

    for b in range(BLOC):
        # ---------- phase A: load + channel pooling ----------
        xt = [[None] * NCH for _ in range(NH)]
        chmax_p = []
        chsum_p = []
        for h in range(NH):
            cmp_t = stat.tile([P, NCH], F32, name=f"chmaxp{b}{h}", tag=f"chmaxp{h}")
            csp_t = stat.tile([P, NCH], F32, name=f"chsump{b}{h}", tag=f"chsump{h}")
            chmax_p.append(cmp_t)
            chsum_p.append(csp_t)
            if "chpool" in skip:
                nc.vector.memset(cmp_t[:], 0.5)
                nc.vector.memset(csp_t[:], 0.5)
        for k in range(NCH):
            for h in range(NH):
                t = xp.tile([P, FCH], F32, name=f"x{b}{h}{k}", tag="x")
                xt[h][k] = t
                nc.sync.dma_start(
                    out=t[:],
                    in_=x_d[b, h * P:(h + 1) * P, k * FCH:(k + 1) * FCH])
                if "chpool" in skip:
                    continue
                nc.vector.tensor_reduce(
                    out=chmax_p[h][:, k:k + 1], in_=t[:],
                    axis=mybir.AxisListType.X, op=mybir.AluOpType.max)
                # in-place copy whose only purpose is the free-dim sum output
                nc.scalar.activation(
                    out=t[:], in_=t[:], func=AF.Copy,
                    accum_out=chsum_p[h][:, k:k + 1])

        # ---------- phase B: channel-attention MLP ----------
        scale_sb = []
        z_ps = mlpp.tile([R, 1], F32, name=f"zps{b}", tag="mlp")
        hvec = []
        for h in range(NH):
            cmf = stat.tile([P, 1], F32, name=f"chmaxf{b}{h}", tag=f"chmaxf{h}")
            csf = stat.tile([P, 1], F32, name=f"chsumf{b}{h}", tag=f"chsumf{h}")
            nc.vector.tensor_reduce(out=cmf[:], in_=chmax_p[h][:],
                                    axis=mybir.AxisListType.X,
                                    op=mybir.AluOpType.max)
            nc.vector.tensor_reduce(out=csf[:], in_=chsum_p[h][:],
                                    axis=mybir.AxisListType.X,
                                    op=mybir.AluOpType.add)
            hv = stat.tile([P, 1], F32, name=f"hvec{b}{h}", tag=f"hvec{h}")
            # hv = chmax + chsum/HW
            nc.scalar.activation(out=hv[:], in_=csf[:], func=AF.Identity,
                                 bias=cmf[:, 0:1], scale=1.0 / HW)
            hvec.append(hv)
        for h in range(NH):
            nc.tensor.matmul(out=z_ps[:], lhsT=w1t_sb[:, h * R:(h + 1) * R],
                             rhs=hvec[h][:], start=(h == 0), stop=(h == NH - 1))
        zr = stat.tile([R, 1], F32, name=f"zrelu{b}", tag="zrelu")
        nc.scalar.activation(out=zr[:], in_=z_ps[:], func=AF.Relu)
        for h in range(NH):
            l_ps = mlpp.tile([P, 1], F32, name=f"lps{b}{h}", tag="mlp")
            nc.tensor.matmul(out=l_ps[:], lhsT=w2t_sb[:, h * P:(h + 1) * P],
                             rhs=zr[:], start=True, stop=True)
            sc = stat.tile([P, 1], F32, name=f"scale{b}{h}", tag=f"scale{h}")
            nc.scalar.activation(out=sc[:], in_=l_ps[:], func=AF.Sigmoid)
            scale_sb.append(sc)

        # ---------- phase C/D/E: scale, spatial stats, conv, final ----------
        smaxT = stat.tile([P, H], F32, name=f"smaxT{b}", tag="smaxT")   # [w, h]
        savgT = stat.tile([P, H], F32, name=f"savgT{b}", tag="savgT")   # [w, h]
        conv_ps = convp.tile([P, H], F32, name=f"convps{b}", tag="conv")
        bcs = {}
        if "trans" in skip:
            nc.vector.memset(smaxT[:], 0.25)
        if "savg" in skip:
            nc.vector.memset(savgT[:], 0.25)

        def conv_pair(g):
            h0c, h1c = g * CONVG * HROWS, (g + 1) * CONVG * HROWS
            # 7x7 conv as banded matmuls: out[:, h] += bandT_{c,i} @ statT[:, h+i-3]
            mms = []
            for c, st in ((0, smaxT), (1, savgT)):
                for i in range(7):
                    lo = max(h0c, 3 - i)
                    hi = min(h1c, H + 3 - i)
                    if lo >= hi:
                        continue
                    mms.append((c, i, lo, hi, st))
            # identity-shift tap first so start=True covers the whole column range
            mms.sort(key=lambda m: (m[1] != 3 or m[0] != 0))
            for n, (c, i, lo, hi, st) in enumerate(mms):
                assert not (n == 0 and (lo != h0c or hi != h1c))
                nc.tensor.matmul(
                    out=conv_ps[:, lo:hi],
                    lhsT=bands_sb[:, (c * 7 + i) * P:(c * 7 + i + 1) * P],
                    rhs=st[:, lo + i - 3:hi + i - 3],
                    start=(n == 0), stop=(n == len(mms) - 1),
                    skip_group_check=True)

        def attn_chunk(kc):
            h0c, h1c = kc * HROWS, (kc + 1) * HROWS
            attn_wh = stat.tile([P, HROWS], F32, name=f"attnwh{b}{kc}",
                                tag="attnwh", bufs=3)
            nc.scalar.activation(out=attn_wh[:], in_=conv_ps[:, h0c:h1c],
                                 func=AF.Sigmoid)
            at_ps = atpp.tile([HROWS, P], F32, name=f"atps{b}{kc}", tag="atp")
            nc.tensor.transpose(out=at_ps[:], in_=attn_wh[:], identity=ident_sb[:])
            attn_hw = stat.tile([HROWS, P], F32, name=f"attnhw{b}{kc}",
                                tag="attnhw", bufs=3)
            nc.scalar.activation(out=attn_hw[:], in_=at_ps[:], func=AF.Copy)
            row = rowp.tile([1, FCH], F32, name=f"row{b}{kc}", tag="row")
            nc.sync.dma_start(
                out=row[:].rearrange("p (h w) -> p h w", h=HROWS),
                in_=attn_hw[:])
            bc = bcp.tile([P, FCH], F32, name=f"bc{b}{kc}", tag="bc")
            nc.gpsimd.partition_broadcast(bc[:], row[:], channels=P)
            bcs[kc] = bc

        def conv_and_final(g):
            if "conv" not in skip:
                conv_pair(g)
            for kc in range(CONVG * g, CONVG * (g + 1)):
                if "conv" not in skip:
                    attn_chunk(kc)
                for h in range(NH):
                    if "final" not in skip and "conv" not in skip:
                        nc.vector.tensor_mul(xt[h][kc][:], xt[h][kc][:],
                                             bcs[kc][:])
                    nc.sync.dma_start(
                        out=out_d[b, h * P:(h + 1) * P,
                                  kc * FCH:(kc + 1) * FCH],
                        in_=xt[h][kc][:])

        for k in range(NCH):
            if "scale" not in skip:
                for h in range(NH):
                    nc.scalar.activation(out=xt[h][k][:], in_=xt[h][k][:],
                                         func=AF.Copy,
                                         scale=scale_sb[h][:, 0:1])
            # spatial max over C: combine halves, transpose, segmented reduce.
            # spatial sum over C: transpose both halves into the same PSUM
            # region with accumulation, then segmented add-reduce.
            if "trans" not in skip:
                xm = xmaxp.tile([P, FCH], F32, name=f"xm{b}{k}", tag="xm")
                nc.vector.tensor_max(xm[:], xt[0][k][:], xt[1][k][:])
                for j4 in range(NBLK // 4):
                    tpt = tp.tile([P, 4 * P], F32, name=f"tp{b}{k}{j4}",
                                  tag="tp")
                    tps = tsp.tile([P, 4 * P], F32, name=f"ts{b}{k}{j4}",
                                   tag="ts")
                    for jj in range(4):
                        j = j4 * 4 + jj
                        nc.tensor.transpose(out=tpt[:, jj * P:(jj + 1) * P],
                                            in_=xm[:, j * P:(j + 1) * P],
                                            identity=ident_sb[:])
                        if "savg" in skip:
                            continue
                        nc.tensor.matmul(out=tps[:, jj * P:(jj + 1) * P],
                                         lhsT=xt[0][k][:, j * P:(j + 1) * P],
                                         rhs=ident_sb[:], is_transpose=True,
                                         start=True, stop=False,
                                         skip_group_check=True)
                        nc.tensor.matmul(out=tps[:, jj * P:(jj + 1) * P],
                                         lhsT=xt[1][k][:, j * P:(j + 1) * P],
                                         rhs=ident_sb[:], is_transpose=True,
                                         start=False, stop=True,
                                         skip_group_check=True)
                    g0 = k * NBLK + j4 * 4
                    nc.vector.tensor_reduce(
                        out=smaxT[:, g0:g0 + 4],
                        in_=tpt[:].rearrange("p (b f) -> p b f", b=4),
                        axis=mybir.AxisListType.X, op=mybir.AluOpType.max)
                    if "savg" not in skip:
                        nc.vector.tensor_reduce(
                            out=savgT[:, g0:g0 + 4],
                            in_=tps[:].rearrange("p (b f) -> p b f", b=4),
                            axis=mybir.AxisListType.X, op=mybir.AluOpType.add)
            if k >= CONVG and k % CONVG == 0:
                conv_and_final((k - CONVG) // CONVG)
        conv_and_final(NCH // CONVG - 1)


def _build_nc(reps: int = 1, skip=frozenset()):
    nc = bacc.Bacc("TRN2", target_bir_lowering=False, debug=False,
                   num_devices=NCORES)
    x_d = nc.dram_tensor("x", [BLOC, C, HW], F32, kind="ExternalInput").ap()
    w1t_d = nc.dram_tensor("w1t", [C, R], F32, kind="ExternalInput").ap()
    w2t_d = nc.dram_tensor("w2t", [R, C], F32, kind="ExternalInput").ap()
    bands_d = nc.dram_tensor("bands", [14, W, W], F32, kind="ExternalInput").ap()
    ident_d = nc.dram_tensor("ident", [P, P], F32, kind="ExternalInput").ap()
    out_d = nc.dram_tensor("out", [BLOC, C, HW], F32, kind="ExternalOutput").ap()
    with tile.TileContext(nc) as tc:
        for _ in range(reps):
            csa_kernel(tc, out_d, x_d, w1t_d, w2t_d, bands_d, ident_d,
                       skip=skip)
    nc.compile()
    return nc


_NC_CACHE = None


def _get_nc():
    global _NC_CACHE
    if _NC_CACHE is None:
        _NC_CACHE = _build_nc()
    return _NC_CACHE


def build_bands(w_conv):
    """[14, W, W] transposed band matrices; bands[c*7+i][w', w] =
    w_conv[0, c, i, w'-w+3]; avg channel folded with 1/C."""
    w_conv = np.asarray(w_conv, np.float32)
    bands = np.zeros((2, 7, W, W), np.float32)
    for c in range(2):
        for i in range(7):
            for kj in range(7):
                bands[c, i] += w_conv[0, c, i, kj] * np.eye(W, k=3 - kj,
                                                            dtype=np.float32)
    bands[1] /= C
    return bands.reshape(14, W, W)


def make_in_maps(x, w_fc1, w_fc2, w_conv):
    x = np.ascontiguousarray(np.asarray(x, np.float32))
    w1t = np.ascontiguousarray(np.asarray(w_fc1, np.float32).T)
    w2t = np.ascontiguousarray(np.asarray(w_fc2, np.float32).T)
    bands = build_bands(w_conv)
    ident = np.eye(P, dtype=np.float32)
    xr = x.reshape(NCORES, BLOC, C, HW)
    return [{"x": np.ascontiguousarray(xr[i]), "w1t": w1t, "w2t": w2t,
             "bands": bands, "ident": ident} for i in range(NCORES)]


def kernel(x, w_fc1, w_fc2, w_conv):
    nc = _get_nc()
    in_maps = make_in_maps(x, w_fc1, w_fc2, w_conv)
    res = run_bass_kernel_spmd(nc, in_maps, list(range(NCORES)))
    out = np.stack([res.results[i]["out"] for i in range(NCORES)])
    return out.reshape(B, C, H, W).astype(np.float32)
